# revision 26
# baseline (speedup 1.0000x reference)
import math
from contextlib import ExitStack

import numpy as np

N, T, D, H = 512, 128, 512, 512
NC = 8
n = N // NC          # 64 samples per core
H4 = 4 * H           # 2048
SCALE = 1.0 / math.sqrt(H)

_cache = {}


def _build_kernel():
    if "nc" in _cache:
        return _cache["nc"]

    import concourse.bass as bass
    import concourse.tile as tile
    from concourse import bacc, mybir

    f32 = mybir.dt.float32
    bf16 = mybir.dt.bfloat16
    ALU = mybir.AluOpType
    ACTF = mybir.ActivationFunctionType

    nc = bacc.Bacc(
        "TRN2",
        target_bir_lowering=False,
        debug=False,
        enable_asserts=False,
        num_devices=NC,
    )

    # ---- DRAM I/O ---------------------------------------------------------
    xTd = nc.dram_tensor("xT", (128, 4 * 8192), bf16, kind="ExternalInput").ap()
    Wxcd = nc.dram_tensor("Wxc", (128, 4 * H4), bf16, kind="ExternalInput").ap()
    Wcd = nc.dram_tensor("Wc", (128, 8 * H4), bf16, kind="ExternalInput").ap()
    ATd = nc.dram_tensor("ATl", (128, 4 * 1024), bf16, kind="ExternalInput").ap()
    Afd = nc.dram_tensor("Af", (128, 8 * 512), bf16, kind="ExternalInput").ap()
    maskTd = nc.dram_tensor("maskT", (128, 512), bf16, kind="ExternalInput").ap()
    id64d = nc.dram_tensor("id64", (n, n), bf16, kind="ExternalInput").ap()
    injWd = nc.dram_tensor("injW", (n + 1, n), bf16, kind="ExternalInput").ap()
    onesd = nc.dram_tensor("onesc", (128, 1), bf16, kind="ExternalInput").ap()
    h0Td = nc.dram_tensor("h0T", (128, 4 * n), bf16, kind="ExternalInput").ap()
    s0d = nc.dram_tensor("s0", (n, H), f32, kind="ExternalInput").ap()
    browd = nc.dram_tensor("brow", (1, H4), bf16, kind="ExternalInput").ap()
    hso = nc.dram_tensor("hso", (T, n, H), bf16, kind="ExternalOutput").ap()

    with tile.TileContext(nc) as tc, ExitStack() as ctx:
        cpool = ctx.enter_context(tc.tile_pool(name="const", bufs=1))
        xtpool = ctx.enter_context(tc.tile_pool(name="xt", bufs=1))
        xwpool = ctx.enter_context(tc.tile_pool(name="xw", bufs=6))
        xbpool = ctx.enter_context(tc.tile_pool(name="xb", bufs=3))
        work = ctx.enter_context(tc.tile_pool(name="work", bufs=3))
        stat = ctx.enter_context(tc.tile_pool(name="stat", bufs=3))
        # PSUM budget (8 banks x 2KB): hb0-3 (4) + of0,of1 (2) + sm (2)
        p_hb = ctx.enter_context(tc.tile_pool(name="p_hb", bufs=1, space="PSUM"))
        p_of = ctx.enter_context(tc.tile_pool(name="p_of", bufs=1, space="PSUM"))
        p_sm = ctx.enter_context(tc.tile_pool(name="p_sm", bufs=2, space="PSUM"))

        # ---- persistent constants ----------------------------------------
        Wxc_sb = cpool.tile([128, 4 * H4], bf16)
        nc.sync.dma_start(Wxc_sb[:], Wxcd[:])
        Wc_sb = cpool.tile([128, 8 * H4], bf16)
        nc.sync.dma_start(Wc_sb[:], Wcd[:])
        AT_sb = cpool.tile([128, 4 * 1024], bf16)
        nc.sync.dma_start(AT_sb[:], ATd[:])
        Af_sb = cpool.tile([128, 8 * 512], bf16)
        nc.sync.dma_start(Af_sb[:], Afd[:])
        maskT_sb = cpool.tile([128, 512], bf16)
        nc.sync.dma_start(maskT_sb[:], maskTd[:])
        id_sb = cpool.tile([n, n], bf16)
        nc.sync.dma_start(id_sb[:], id64d[:])
        injW_sb = cpool.tile([n + 1, n], bf16)
        nc.sync.dma_start(injW_sb[:], injWd[:])
        ones_sb = cpool.tile([128, 1], bf16)
        nc.sync.dma_start(ones_sb[:], onesd[:])
        hTa = cpool.tile([128, 2 * n], bf16)
        nc.sync.dma_start(hTa[:], h0Td[:, 0:2 * n])
        hTb = cpool.tile([128, 2 * n], bf16)
        nc.sync.dma_start(hTb[:], h0Td[:, 2 * n:4 * n])
        s_st = cpool.tile([n, H], f32)
        nc.sync.dma_start(s_st[:], s0d[:])

        def hTk(k):  # chunk k of the (2h)^T state
            return (hTa, hTb)[k // 2][:, n * (k % 2):n * (k % 2 + 1)]

        # ---- phase 1: xwx[r, :] = x_flat[r, :] @ Wx -----------------------
        # xT loaded in 8 column pieces so matmuls start after the first piece
        xT_sb = xtpool.tile([128, 4 * 8192], bf16)
        for p in range(8):
            for k in range(4):
                nc.sync.dma_start(
                    xT_sb[:, 8192 * k + 1024 * p:8192 * k + 1024 * (p + 1)],
                    xTd[:, 8192 * k + 1024 * p:8192 * k + 1024 * (p + 1)])

        # tiles 0-3 are produced up front; tiles 4-63 are interleaved into
        # the recurrence (two quarters per step) to fill PE idle slots
        xw_tiles = {}

        def xwq(m, q, pool, psum_tag, use_act):
            """One 512-col quarter of xwx tile m: 4 matmuls + psum->sbuf copy;
            DMA the full tile out after its last quarter."""
            if m not in xw_tiles:
                xw_tiles[m] = xwpool.tile([128, H4], bf16, tag="xw",
                                          name=f"xw{m}")
            xw_sb = xw_tiles[m]
            ps = pool.tile([128, 512], f32, tag=psum_tag, name=f"ps{m}_{q}")
            for k in range(4):
                lhs = xT_sb[:, 8192 * k + 128 * m:8192 * k + 128 * (m + 1)]
                nc.tensor.matmul(
                    ps[:],
                    lhs,
                    Wxc_sb[:, H4 * k + 512 * q:H4 * k + 512 * (q + 1)],
                    start=(k == 0),
                    stop=(k == 3),
                )
            if use_act:
                nc.scalar.activation(xw_sb[:, 512 * q:512 * (q + 1)],
                                     ps[:], ACTF.Copy)
            else:
                nc.vector.tensor_copy(xw_sb[:, 512 * q:512 * (q + 1)], ps[:])


        for m in range(4):
            for q in range(4):
                xwq(m, q, p_hb, f"hb{q}", q % 2 == 0)

        # ---- loop prologue ------------------------------------------------
        xq = {}

        def prefetch(s):
            # stage rows for step s from the SBUF xwx tile (tracked dep),
            # with the bias row appended for the K=65 inject
            xq[s] = xbpool.tile([n + 1, H4], bf16, tag="xq", name=f"xq{s}")
            nc.sync.dma_start(
                xq[s][0:n], xw_tiles[s // 2][n * (s % 2):n * (s % 2 + 1), :])
            nc.sync.dma_start(xq[s][n:n + 1], browd[:])

        for s in (0, 1, 2):
            prefetch(s)

        def make_hb(t):
            return [p_hb.tile([n, 512], f32, tag=f"hb{j}", name=f"hb{t}_{j}")
                    for j in range(4)]

        def inject(hbl, xqt):
            for j in range(4):
                nc.tensor.matmul(
                    hbl[j][:],
                    injW_sb[:],
                    xqt[:, 512 * j:512 * (j + 1)],
                    start=True,
                    stop=False,
                )

        hbl = make_hb(0)
        inject(hbl, xq[0])

        # ---- recurrence ---------------------------------------------------
        # state: hT = (2h)^T bf16, s_st = 2c f32.  AT and the Wh half of Wc
        # are pre-scaled by 0.5 on the host; hso holds 2h (host halves it).
        # gate column order in all 2048-wide tensors is [i, f, g, o].
        for t in range(T):
            if t + 3 < T:
                prefetch(t + 3)

            # out_full[s, (s',p)] = sum_h h[s,h] A_flat[s',p,h]
            of = [None, None]
            for g in range(2):
                of[g] = p_of.tile([n, 512], f32, tag=f"of{g}", name=f"of{t}_{g}")
                for k in range(4):
                    nc.tensor.matmul(
                        of[g][:],
                        hTk(k),
                        AT_sb[:, 1024 * k + 512 * g:1024 * k + 512 * (g + 1)],
                        start=(k == 0),
                        stop=(k == 3),
                    )

            # h-part of main matmul chunks 0-1 (fills PE during ACT/DVE work)
            for k in range(2):
                for j in range(4):
                    nc.tensor.matmul(
                        hbl[j][:],
                        hTk(k),
                        Wc_sb[:, H4 * k + 512 * j:H4 * k + 512 * (j + 1)],
                        start=False,
                        stop=False,
                    )

            # exp (scale folded in; no max-subtraction needed)
            wexp = [None, None]
            for g in range(2):
                wexp[g] = work.tile([n, 512], bf16, tag=f"we{g}",
                                    name=f"we{t}_{g}")
                nc.scalar.activation(wexp[g][:], of[g][:], ACTF.Exp,
                                     scale=SCALE)

            # transpose wexp -> [(s',p) chunks, s]; mask applied in the copy
            wT = [None, None]
            wexpT = [None, None]
            for g in range(2):
                wT[g] = p_sm.tile([128, 256], bf16, tag="sm", name=f"wT{t}_{g}")
                for cc in range(4):
                    nc.tensor.transpose(
                        wT[g][:, n * cc:n * (cc + 1)],
                        wexp[g][:, 128 * cc:128 * (cc + 1)],
                        id_sb[:])
                wexpT[g] = work.tile([128, 256], bf16, tag=f"wexpT{g}",
                                     name=f"wT2{t}_{g}")
                nc.vector.tensor_tensor(
                    wexpT[g][:], wT[g][:],
                    maskT_sb[:, 256 * g:256 * (g + 1)], ALU.mult)

            # h-part chunk 2
            for j in range(4):
                nc.tensor.matmul(
                    hbl[j][:],
                    hTk(2),
                    Wc_sb[:, H4 * 2 + 512 * j:H4 * 2 + 512 * (j + 1)],
                    start=False, stop=False)

            # attn (unnormalized) = wexp_masked @ A_flat; ss = row sums via
            # a ones column (same stationary weights)
            aa = p_sm.tile([n, 512], f32, tag="sm", name=f"aa{t}")
            ss = p_of.tile([n, 1], f32, tag="of0", name=f"ss{t}")
            for cch in range(8):
                wcol = wexpT[cch // 4][:, n * (cch % 4):n * (cch % 4 + 1)]
                nc.tensor.matmul(
                    aa[:], wcol, Af_sb[:, 512 * cch:512 * (cch + 1)],
                    start=(cch == 0), stop=(cch == 7))
                nc.tensor.matmul(
                    ss[:], wcol, ones_sb[:],
                    start=(cch == 0), stop=(cch == 7))

            # h-part chunk 3
            for j in range(4):
                nc.tensor.matmul(
                    hbl[j][:],
                    hTk(3),
                    Wc_sb[:, H4 * 3 + 512 * j:H4 * 3 + 512 * (j + 1)],
                    start=False, stop=False)

            rinv = stat.tile([n, 1], f32, tag="rinv", name=f"ri{t}")
            nc.vector.reciprocal(rinv[:], ss[:])

            # normalize by 1/ssum during PSUM->SBUF copy (per-partition scale)
            attn_n = work.tile([n, H], bf16, tag="attn_n", name=f"an{t}")
            nc.scalar.activation(attn_n[:], aa[:], ACTF.Copy, scale=rinv[:])

            # transpose attn -> attnT
            at = p_sm.tile([128, 4 * n], bf16, tag="sm", name=f"at{t}")
            for k in range(4):
                nc.tensor.transpose(
                    at[:, n * k:n * (k + 1)],
                    attn_n[:, 128 * k:128 * (k + 1)],
                    id_sb[:])
            attnT = work.tile([128, 4 * n], bf16, tag="attnT", name=f"aT{t}")
            nc.vector.tensor_copy(attnT[:], at[:])

            # attn-part group-major (per-gate psum tiles complete early);
            # group order f, i, g, o matches the c-path dependency chain.
            # sigmoid(x) = (tanh(x/2)+1)/2 folded into the 2h/2c state.
            def attn_part(j):
                for k in range(4):
                    nc.tensor.matmul(
                        hbl[j][:],
                        attnT[:, n * k:n * (k + 1)],
                        Wc_sb[:, H4 * (4 + k) + 512 * j:H4 * (4 + k) + 512 * (j + 1)],
                        start=False,
                        stop=(k == 3),
                    )

            t_i = work.tile([n, H], f32, tag="t_i", name=f"ti{t}")
            t_f = work.tile([n, H], f32, tag="t_f", name=f"tf{t}")
            u1 = work.tile([n, H], f32, tag="u1", name=f"u1{t}")
            # tail ops run in 256-col halves so ACT/DVE/PE pipeline the
            # g -> u2 -> s -> tanh(c) -> h chain
            t_g = [work.tile([n, 256], f32, tag=f"tg{z}", name=f"tg{t}_{z}")
                   for z in range(2)]
            t_o = [work.tile([n, 256], f32, tag=f"to{z}", name=f"to{t}_{z}")
                   for z in range(2)]
            u2 = [work.tile([n, 256], f32, tag=f"u2{z}", name=f"u2{t}_{z}")
                  for z in range(2)]
            ct = [work.tile([n, 256], f32, tag=f"ct{z}", name=f"ct{t}_{z}")
                  for z in range(2)]
            h_bf = [work.tile([n, 256], bf16, tag=f"h{z}", name=f"h{t}_{z}")
                    for z in range(2)]

            attn_part(1)  # f
            nc.scalar.activation(t_f[:], hbl[1][:], ACTF.Tanh, scale=0.5)
            # u1 = (t_f+1)*s_prev = 4 sig_f c
            nc.vector.scalar_tensor_tensor(
                u1[:], t_f[:], 1.0, s_st[:], ALU.add, ALU.mult)
            attn_part(0)  # i
            nc.scalar.activation(t_i[:], hbl[0][:], ACTF.Tanh, scale=0.5)
            attn_part(2)  # g
            attn_part(3)  # o

            # interleaved xwx production (tiles 4-63), both quarters in the
            # tail where PE otherwise idles on the c->h chain
            qi = 2 * t
            if qi < 240:
                xwq(4 + qi // 4, qi % 4, p_of, "of0", True)
            qi = 2 * t + 1
            if qi < 240:
                xwq(4 + qi // 4, qi % 4, p_of, "of1", False)

            for z in range(2):
                cz = slice(256 * z, 256 * (z + 1))
                nc.scalar.activation(t_g[z][:], hbl[2][:, cz], ACTF.Tanh)
                # u2 = (t_i+1)*g = 2 sig_i g
                nc.vector.scalar_tensor_tensor(
                    u2[z][:], t_i[:, cz], 1.0, t_g[z][:], ALU.add, ALU.mult)
                nc.scalar.activation(t_o[z][:], hbl[3][:, cz], ACTF.Tanh,
                                     scale=0.5)
                # s = 0.5*u1 + u2 = 2c
                nc.vector.scalar_tensor_tensor(
                    s_st[:, cz], u1[:, cz], 0.5, u2[z][:], ALU.mult, ALU.add)
                nc.scalar.activation(ct[z][:], s_st[:, cz], ACTF.Tanh,
                                     scale=0.5)
                # h2 = 2h = (t_o+1)*tanh(c)
                nc.vector.scalar_tensor_tensor(
                    h_bf[z][:], t_o[z][:], 1.0, ct[z][:], ALU.add, ALU.mult)
                nc.sync.dma_start(hso[t][:, cz], h_bf[z][:])

                # hT for next step: chunks 2z, 2z+1 come from half z
                if t + 1 < T:
                    hTp = p_sm.tile([128, 2 * n], bf16, tag="sm",
                                    name=f"hTp{t}_{z}")
                    for kk in range(2):
                        nc.tensor.transpose(
                            hTp[:, n * kk:n * (kk + 1)],
                            h_bf[z][:, 128 * kk:128 * (kk + 1)],
                            id_sb[:])
                    nc.vector.tensor_copy((hTa, hTb)[z][:], hTp[:])

            if t + 1 < T:
                hbl = make_hb(t + 1)
                inject(hbl, xq[t + 1])

    nc.compile()
    _cache["nc"] = nc
    return nc


def _prep_host(x, A, Wx, Wh, Wattn, b):
    import ml_dtypes
    bft = ml_dtypes.bfloat16

    # gate column order [i, f, g, o] (reference is [i, f, o, g])
    perm = np.concatenate([
        np.arange(0, 1024),
        np.arange(1536, 2048),
        np.arange(1024, 1536),
    ])
    Wxp = np.asarray(Wx, np.float32)[:, perm]
    # Wh scaled by 0.5: the h state on device is 2h
    Wcat = np.concatenate(
        [0.5 * np.asarray(Wh, np.float32)[:, perm],
         np.asarray(Wattn, np.float32)[:, perm]], axis=0)     # (1024, 2048)
    bp = np.asarray(b, np.float32)[perm]

    Wc_host = np.ascontiguousarray(
        Wcat.reshape(8, 128, H4).transpose(1, 0, 2).reshape(128, 8 * H4)
    ).astype(bft)
    Wxc_host = np.ascontiguousarray(
        Wxp.reshape(4, 128, H4).transpose(1, 0, 2).reshape(128, 4 * H4)
    ).astype(bft)
    brow = bp.reshape(1, H4).astype(bft)
    id64 = np.eye(n, dtype=np.float32).astype(bft)
    injW = np.concatenate(
        [np.eye(n, dtype=np.float32),
         np.ones((1, n), np.float32)], axis=0).astype(bft)
    onesc = np.ones((128, 1), np.float32).astype(bft)
    # maskT[p, 64c+s] = 1 if (128c+p)//16 == s
    r = np.arange(1024)
    maskbd = (r[:, None] // 16 == np.arange(n)[None, :]).astype(np.float32)
    maskT = np.ascontiguousarray(
        maskbd.reshape(8, 128, n).transpose(1, 0, 2).reshape(128, 8 * n)
    ).astype(bft)

    in_maps = []
    for k in range(NC):
        xc = np.asarray(x[n * k:n * (k + 1)], np.float32)     # (64, T, D)
        Ac = np.asarray(A[n * k:n * (k + 1)], np.float32)     # (64, H, 4, 4)

        x_flat = xc.transpose(1, 0, 2).reshape(T * n, D)      # r = t*64+s
        xT_host = np.ascontiguousarray(
            x_flat.T.reshape(4, 128, T * n).transpose(1, 0, 2)
            .reshape(128, 4 * T * n)).astype(bft)

        A_flat = Ac.reshape(n, H, 16).transpose(0, 2, 1)      # (64, 16, H)
        A_rows = np.ascontiguousarray(A_flat.reshape(n * 16, H))
        # AT scaled by 0.5 (h state is 2h)
        AT_host = np.ascontiguousarray(
            (0.5 * A_rows.T).reshape(4, 128, 1024).transpose(1, 0, 2)
            .reshape(128, 4 * 1024)).astype(bft)
        Af_host = np.ascontiguousarray(
            A_rows.reshape(8, 128, H).transpose(1, 0, 2)
            .reshape(128, 8 * H)).astype(bft)

        h0 = Ac.mean(axis=(2, 3)).astype(np.float32)          # (64, 512)
        h0T_host = np.ascontiguousarray(
            (2.0 * h0.T).reshape(4, 128, n).transpose(1, 0, 2)
            .reshape(128, 4 * n)).astype(bft)

        in_maps.append({
            "xT": xT_host,
            "Wxc": Wxc_host,
            "Wc": Wc_host,
            "ATl": AT_host,
            "Af": Af_host,
            "maskT": maskT,
            "id64": id64,
            "injW": injW,
            "onesc": onesc,
            "h0T": h0T_host,
            "s0": (2.0 * h0).astype(np.float32),
            "brow": brow,
        })
    return in_maps


def kernel(x, A, Wx, Wh, Wattn, b):
    from concourse import bass_utils

    nc = _build_kernel()
    in_maps = _prep_host(x, A, Wx, Wh, Wattn, b)
    res = bass_utils.run_bass_kernel_spmd(nc, in_maps, core_ids=list(range(NC)))

    out = np.empty((N, T, H), dtype=np.float32)
    for k in range(NC):
        hs_k = np.asarray(res.results[k]["hso"]).astype(np.float32)  # (T, n, H)
        out[n * k:n * (k + 1)] = 0.5 * hs_k.transpose(1, 0, 2)
    return out


# revision 29
# speedup vs baseline: 1.0011x; 1.0011x over previous
import math
from contextlib import ExitStack

import numpy as np

N, T, D, H = 512, 128, 512, 512
NC = 8
n = N // NC          # 64 samples per core
H4 = 4 * H           # 2048
SCALE = 1.0 / math.sqrt(H)

_cache = {}


def _build_kernel():
    if "nc" in _cache:
        return _cache["nc"]

    import concourse.bass as bass
    import concourse.tile as tile
    from concourse import bacc, mybir

    f32 = mybir.dt.float32
    bf16 = mybir.dt.bfloat16
    ALU = mybir.AluOpType
    ACTF = mybir.ActivationFunctionType

    nc = bacc.Bacc(
        "TRN2",
        target_bir_lowering=False,
        debug=False,
        enable_asserts=False,
        num_devices=NC,
    )

    # ---- DRAM I/O ---------------------------------------------------------
    xTd = nc.dram_tensor("xT", (128, 4 * 8192), bf16, kind="ExternalInput").ap()
    Wxcd = nc.dram_tensor("Wxc", (128, 4 * H4), bf16, kind="ExternalInput").ap()
    Wcd = nc.dram_tensor("Wc", (128, 8 * H4), bf16, kind="ExternalInput").ap()
    ATd = nc.dram_tensor("ATl", (128, 4 * 1024), bf16, kind="ExternalInput").ap()
    Afd = nc.dram_tensor("Af", (128, 8 * 512), bf16, kind="ExternalInput").ap()
    maskTd = nc.dram_tensor("maskT", (128, 512), bf16, kind="ExternalInput").ap()
    id64d = nc.dram_tensor("id64", (n, n), bf16, kind="ExternalInput").ap()
    injWd = nc.dram_tensor("injW", (n + 1, n), bf16, kind="ExternalInput").ap()
    onesd = nc.dram_tensor("onesc", (128, 1), bf16, kind="ExternalInput").ap()
    h0Td = nc.dram_tensor("h0T", (128, 4 * n), bf16, kind="ExternalInput").ap()
    s0d = nc.dram_tensor("s0", (n, H), f32, kind="ExternalInput").ap()
    browd = nc.dram_tensor("brow", (1, H4), bf16, kind="ExternalInput").ap()
    hso = nc.dram_tensor("hso", (T, n, H), bf16, kind="ExternalOutput").ap()

    with tile.TileContext(nc) as tc, ExitStack() as ctx:
        cpool = ctx.enter_context(tc.tile_pool(name="const", bufs=1))
        xtpool = ctx.enter_context(tc.tile_pool(name="xt", bufs=1))
        xwpool = ctx.enter_context(tc.tile_pool(name="xw", bufs=6))
        xbpool = ctx.enter_context(tc.tile_pool(name="xb", bufs=3))
        work = ctx.enter_context(tc.tile_pool(name="work", bufs=3))
        stat = ctx.enter_context(tc.tile_pool(name="stat", bufs=3))
        # PSUM budget (8 banks x 2KB): hb0-3 (4) + of0,of1 (2) + sm (2)
        p_hb = ctx.enter_context(tc.tile_pool(name="p_hb", bufs=1, space="PSUM"))
        p_of = ctx.enter_context(tc.tile_pool(name="p_of", bufs=1, space="PSUM"))
        p_sm = ctx.enter_context(tc.tile_pool(name="p_sm", bufs=2, space="PSUM"))

        # ---- persistent constants ----------------------------------------
        Wxc_sb = cpool.tile([128, 4 * H4], bf16)
        nc.sync.dma_start(Wxc_sb[:], Wxcd[:])
        Wc_sb = cpool.tile([128, 8 * H4], bf16)
        nc.sync.dma_start(Wc_sb[:], Wcd[:])
        AT_sb = cpool.tile([128, 4 * 1024], bf16)
        nc.sync.dma_start(AT_sb[:], ATd[:])
        Af_sb = cpool.tile([128, 8 * 512], bf16)
        nc.sync.dma_start(Af_sb[:], Afd[:])
        maskT_sb = cpool.tile([128, 512], bf16)
        nc.sync.dma_start(maskT_sb[:], maskTd[:])
        id_sb = cpool.tile([n, n], bf16)
        nc.sync.dma_start(id_sb[:], id64d[:])
        injW_sb = cpool.tile([n + 1, n], bf16)
        nc.sync.dma_start(injW_sb[:], injWd[:])
        ones_sb = cpool.tile([128, 1], bf16)
        nc.sync.dma_start(ones_sb[:], onesd[:])
        hTa = cpool.tile([128, 2 * n], bf16)
        nc.sync.dma_start(hTa[:], h0Td[:, 0:2 * n])
        hTb = cpool.tile([128, 2 * n], bf16)
        nc.sync.dma_start(hTb[:], h0Td[:, 2 * n:4 * n])
        s_st = cpool.tile([n, H], f32)
        nc.sync.dma_start(s_st[:], s0d[:])

        def hTk(k):  # chunk k of the (2h)^T state
            return (hTa, hTb)[k // 2][:, n * (k % 2):n * (k % 2 + 1)]

        # ---- phase 1: xwx[r, :] = x_flat[r, :] @ Wx -----------------------
        # xT loaded in 8 column pieces so matmuls start after the first piece
        xT_sb = xtpool.tile([128, 4 * 8192], bf16)
        for p in range(8):
            for k in range(4):
                nc.sync.dma_start(
                    xT_sb[:, 8192 * k + 1024 * p:8192 * k + 1024 * (p + 1)],
                    xTd[:, 8192 * k + 1024 * p:8192 * k + 1024 * (p + 1)])

        # tiles 0-3 are produced up front; tiles 4-63 are interleaved into
        # the recurrence (two quarters per step) to fill PE idle slots
        xw_tiles = {}

        def xwq_mm(m, q, pool, psum_tag):
            """Matmul part of one 512-col xwx quarter (copy deferred)."""
            if m not in xw_tiles:
                xw_tiles[m] = xwpool.tile([128, H4], bf16, tag="xw",
                                          name=f"xw{m}")
            ps = pool.tile([128, 512], f32, tag=psum_tag, name=f"ps{m}_{q}")
            for k in range(4):
                lhs = xT_sb[:, 8192 * k + 128 * m:8192 * k + 128 * (m + 1)]
                nc.tensor.matmul(
                    ps[:],
                    lhs,
                    Wxc_sb[:, H4 * k + 512 * q:H4 * k + 512 * (q + 1)],
                    start=(k == 0),
                    stop=(k == 3),
                )
            return ps, xw_tiles[m], q

        def xwq_copy(ps, xw_sb, q, use_act):
            if use_act:
                nc.scalar.activation(xw_sb[:, 512 * q:512 * (q + 1)],
                                     ps[:], ACTF.Copy)
            else:
                nc.vector.tensor_copy(xw_sb[:, 512 * q:512 * (q + 1)], ps[:])

        def xwq(m, q, pool, psum_tag, use_act):
            xwq_copy(*xwq_mm(m, q, pool, psum_tag), use_act)


        for m in range(4):
            for q in range(4):
                xwq(m, q, p_hb, f"hb{q}", q % 2 == 0)

        # ---- loop prologue ------------------------------------------------
        xq = {}

        def prefetch(s):
            # stage rows for step s from the SBUF xwx tile (tracked dep),
            # with the bias row appended for the K=65 inject
            xq[s] = xbpool.tile([n + 1, H4], bf16, tag="xq", name=f"xq{s}")
            nc.sync.dma_start(
                xq[s][0:n], xw_tiles[s // 2][n * (s % 2):n * (s % 2 + 1), :])
            nc.sync.dma_start(xq[s][n:n + 1], browd[:])

        for s in (0, 1, 2):
            prefetch(s)

        def make_hb(t):
            return [p_hb.tile([n, 512], f32, tag=f"hb{j}", name=f"hb{t}_{j}")
                    for j in range(4)]

        def inject(hbl, xqt):
            for j in range(4):
                nc.tensor.matmul(
                    hbl[j][:],
                    injW_sb[:],
                    xqt[:, 512 * j:512 * (j + 1)],
                    start=True,
                    stop=False,
                )

        hbl = make_hb(0)
        inject(hbl, xq[0])

        # ---- recurrence ---------------------------------------------------
        # state: hT = (2h)^T bf16, s_st = 2c f32.  AT and the Wh half of Wc
        # are pre-scaled by 0.5 on the host; hso holds 2h (host halves it).
        # gate column order in all 2048-wide tensors is [i, f, g, o].
        for t in range(T):
            if t + 3 < T:
                prefetch(t + 3)

            # out_full[s, (s',p)] = sum_h h[s,h] A_flat[s',p,h]
            of = [None, None]
            for g in range(2):
                of[g] = p_of.tile([n, 512], f32, tag=f"of{g}", name=f"of{t}_{g}")
                for k in range(4):
                    nc.tensor.matmul(
                        of[g][:],
                        hTk(k),
                        AT_sb[:, 1024 * k + 512 * g:1024 * k + 512 * (g + 1)],
                        start=(k == 0),
                        stop=(k == 3),
                    )

            # h-part of main matmul chunks 0-1 (fills PE during ACT/DVE work)
            for k in range(2):
                for j in range(4):
                    nc.tensor.matmul(
                        hbl[j][:],
                        hTk(k),
                        Wc_sb[:, H4 * k + 512 * j:H4 * k + 512 * (j + 1)],
                        start=False,
                        stop=False,
                    )

            # exp (scale folded in; no max-subtraction needed)
            wexp = [None, None]
            for g in range(2):
                wexp[g] = work.tile([n, 512], bf16, tag=f"we{g}",
                                    name=f"we{t}_{g}")
                nc.scalar.activation(wexp[g][:], of[g][:], ACTF.Exp,
                                     scale=SCALE)

            # transpose wexp -> [(s',p) chunks, s]; mask applied in the copy
            wT = [None, None]
            wexpT = [None, None]
            for g in range(2):
                wT[g] = p_sm.tile([128, 256], bf16, tag="sm", name=f"wT{t}_{g}")
                for cc in range(4):
                    nc.tensor.transpose(
                        wT[g][:, n * cc:n * (cc + 1)],
                        wexp[g][:, 128 * cc:128 * (cc + 1)],
                        id_sb[:])
                wexpT[g] = work.tile([128, 256], bf16, tag=f"wexpT{g}",
                                     name=f"wT2{t}_{g}")
                nc.vector.tensor_tensor(
                    wexpT[g][:], wT[g][:],
                    maskT_sb[:, 256 * g:256 * (g + 1)], ALU.mult)

            # h-part chunk 2
            for j in range(4):
                nc.tensor.matmul(
                    hbl[j][:],
                    hTk(2),
                    Wc_sb[:, H4 * 2 + 512 * j:H4 * 2 + 512 * (j + 1)],
                    start=False, stop=False)

            # attn (unnormalized) = wexp_masked @ A_flat; ss = row sums via
            # a ones column (same stationary weights)
            aa = p_sm.tile([n, 512], f32, tag="sm", name=f"aa{t}")
            ss = p_of.tile([n, 1], f32, tag="of0", name=f"ss{t}")
            for cch in range(8):
                wcol = wexpT[cch // 4][:, n * (cch % 4):n * (cch % 4 + 1)]
                nc.tensor.matmul(
                    aa[:], wcol, Af_sb[:, 512 * cch:512 * (cch + 1)],
                    start=(cch == 0), stop=(cch == 7))
                nc.tensor.matmul(
                    ss[:], wcol, ones_sb[:],
                    start=(cch == 0), stop=(cch == 7))

            # h-part chunk 3
            for j in range(4):
                nc.tensor.matmul(
                    hbl[j][:],
                    hTk(3),
                    Wc_sb[:, H4 * 3 + 512 * j:H4 * 3 + 512 * (j + 1)],
                    start=False, stop=False)

            rinv = stat.tile([n, 1], f32, tag="rinv", name=f"ri{t}")
            nc.vector.reciprocal(rinv[:], ss[:])

            # normalize by 1/ssum during PSUM->SBUF copy (per-partition scale)
            attn_n = work.tile([n, H], bf16, tag="attn_n", name=f"an{t}")
            nc.scalar.activation(attn_n[:], aa[:], ACTF.Copy, scale=rinv[:])

            # transpose attn -> attnT
            at = p_sm.tile([128, 4 * n], bf16, tag="sm", name=f"at{t}")
            for k in range(4):
                nc.tensor.transpose(
                    at[:, n * k:n * (k + 1)],
                    attn_n[:, 128 * k:128 * (k + 1)],
                    id_sb[:])
            attnT = work.tile([128, 4 * n], bf16, tag="attnT", name=f"aT{t}")
            nc.vector.tensor_copy(attnT[:], at[:])

            # attn-part group-major (per-gate psum tiles complete early);
            # group order f, i, g, o matches the c-path dependency chain.
            # sigmoid(x) = (tanh(x/2)+1)/2 folded into the 2h/2c state.
            def attn_part(j):
                for k in range(4):
                    nc.tensor.matmul(
                        hbl[j][:],
                        attnT[:, n * k:n * (k + 1)],
                        Wc_sb[:, H4 * (4 + k) + 512 * j:H4 * (4 + k) + 512 * (j + 1)],
                        start=False,
                        stop=(k == 3),
                    )

            t_i = work.tile([n, H], f32, tag="t_i", name=f"ti{t}")
            t_f = work.tile([n, H], f32, tag="t_f", name=f"tf{t}")
            u1 = work.tile([n, H], f32, tag="u1", name=f"u1{t}")
            # tail ops run in 256-col halves so ACT/DVE/PE pipeline the
            # g -> u2 -> s -> tanh(c) -> h chain
            t_g = [work.tile([n, 256], f32, tag=f"tg{z}", name=f"tg{t}_{z}")
                   for z in range(2)]
            t_o = [work.tile([n, 256], f32, tag=f"to{z}", name=f"to{t}_{z}")
                   for z in range(2)]
            u2 = [work.tile([n, 256], f32, tag=f"u2{z}", name=f"u2{t}_{z}")
                  for z in range(2)]
            ct = [work.tile([n, 256], f32, tag=f"ct{z}", name=f"ct{t}_{z}")
                  for z in range(2)]
            h_bf = [work.tile([n, 256], bf16, tag=f"h{z}", name=f"h{t}_{z}")
                    for z in range(2)]

            attn_part(1)  # f
            nc.scalar.activation(t_f[:], hbl[1][:], ACTF.Tanh, scale=0.5)
            # u1 = (t_f+1)*s_prev = 4 sig_f c
            nc.vector.scalar_tensor_tensor(
                u1[:], t_f[:], 1.0, s_st[:], ALU.add, ALU.mult)
            attn_part(0)  # i
            nc.scalar.activation(t_i[:], hbl[0][:], ACTF.Tanh, scale=0.5)
            attn_part(2)  # g
            attn_part(3)  # o

            # interleaved xwx production (tiles 4-63): matmuls fill the PE
            # idle in the tail; the psum->sbuf copies are deferred below the
            # c->h chain so they don't delay it in the ACT/DVE queues
            pend = []
            qi = 2 * t
            if qi < 240:
                pend.append(xwq_mm(4 + qi // 4, qi % 4, p_of, "of0"))
            qi = 2 * t + 1
            if qi < 240:
                pend.append(xwq_mm(4 + qi // 4, qi % 4, p_of, "of1"))

            for z in range(2):
                cz = slice(256 * z, 256 * (z + 1))
                nc.scalar.activation(t_g[z][:], hbl[2][:, cz], ACTF.Tanh)
                # u2 = (t_i+1)*g = 2 sig_i g
                nc.vector.scalar_tensor_tensor(
                    u2[z][:], t_i[:, cz], 1.0, t_g[z][:], ALU.add, ALU.mult)
                nc.scalar.activation(t_o[z][:], hbl[3][:, cz], ACTF.Tanh,
                                     scale=0.5)
                # s = 0.5*u1 + u2 = 2c
                nc.vector.scalar_tensor_tensor(
                    s_st[:, cz], u1[:, cz], 0.5, u2[z][:], ALU.mult, ALU.add)
                nc.scalar.activation(ct[z][:], s_st[:, cz], ACTF.Tanh,
                                     scale=0.5)
                # h2 = 2h = (t_o+1)*tanh(c)
                nc.vector.scalar_tensor_tensor(
                    h_bf[z][:], t_o[z][:], 1.0, ct[z][:], ALU.add, ALU.mult)
                nc.sync.dma_start(hso[t][:, cz], h_bf[z][:])

                # hT for next step: chunks 2z, 2z+1 come from half z
                if t + 1 < T:
                    hTp = p_sm.tile([128, 2 * n], bf16, tag="sm",
                                    name=f"hTp{t}_{z}")
                    for kk in range(2):
                        nc.tensor.transpose(
                            hTp[:, n * kk:n * (kk + 1)],
                            h_bf[z][:, 128 * kk:128 * (kk + 1)],
                            id_sb[:])
                    nc.vector.tensor_copy((hTa, hTb)[z][:], hTp[:])

            for i, pq in enumerate(pend):
                xwq_copy(*pq, i == 0)

            if t + 1 < T:
                hbl = make_hb(t + 1)
                inject(hbl, xq[t + 1])

    nc.compile()
    _cache["nc"] = nc
    return nc


def _prep_host(x, A, Wx, Wh, Wattn, b):
    import ml_dtypes
    bft = ml_dtypes.bfloat16

    # gate column order [i, f, g, o] (reference is [i, f, o, g])
    perm = np.concatenate([
        np.arange(0, 1024),
        np.arange(1536, 2048),
        np.arange(1024, 1536),
    ])
    Wxp = np.asarray(Wx, np.float32)[:, perm]
    # Wh scaled by 0.5: the h state on device is 2h
    Wcat = np.concatenate(
        [0.5 * np.asarray(Wh, np.float32)[:, perm],
         np.asarray(Wattn, np.float32)[:, perm]], axis=0)     # (1024, 2048)
    bp = np.asarray(b, np.float32)[perm]

    Wc_host = np.ascontiguousarray(
        Wcat.reshape(8, 128, H4).transpose(1, 0, 2).reshape(128, 8 * H4)
    ).astype(bft)
    Wxc_host = np.ascontiguousarray(
        Wxp.reshape(4, 128, H4).transpose(1, 0, 2).reshape(128, 4 * H4)
    ).astype(bft)
    brow = bp.reshape(1, H4).astype(bft)
    id64 = np.eye(n, dtype=np.float32).astype(bft)
    injW = np.concatenate(
        [np.eye(n, dtype=np.float32),
         np.ones((1, n), np.float32)], axis=0).astype(bft)
    onesc = np.ones((128, 1), np.float32).astype(bft)
    # maskT[p, 64c+s] = 1 if (128c+p)//16 == s
    r = np.arange(1024)
    maskbd = (r[:, None] // 16 == np.arange(n)[None, :]).astype(np.float32)
    maskT = np.ascontiguousarray(
        maskbd.reshape(8, 128, n).transpose(1, 0, 2).reshape(128, 8 * n)
    ).astype(bft)

    in_maps = []
    for k in range(NC):
        xc = np.asarray(x[n * k:n * (k + 1)], np.float32)     # (64, T, D)
        Ac = np.asarray(A[n * k:n * (k + 1)], np.float32)     # (64, H, 4, 4)

        x_flat = xc.transpose(1, 0, 2).reshape(T * n, D)      # r = t*64+s
        xT_host = np.ascontiguousarray(
            x_flat.T.reshape(4, 128, T * n).transpose(1, 0, 2)
            .reshape(128, 4 * T * n)).astype(bft)

        A_flat = Ac.reshape(n, H, 16).transpose(0, 2, 1)      # (64, 16, H)
        A_rows = np.ascontiguousarray(A_flat.reshape(n * 16, H))
        # AT scaled by 0.5 (h state is 2h)
        AT_host = np.ascontiguousarray(
            (0.5 * A_rows.T).reshape(4, 128, 1024).transpose(1, 0, 2)
            .reshape(128, 4 * 1024)).astype(bft)
        Af_host = np.ascontiguousarray(
            A_rows.reshape(8, 128, H).transpose(1, 0, 2)
            .reshape(128, 8 * H)).astype(bft)

        h0 = Ac.mean(axis=(2, 3)).astype(np.float32)          # (64, 512)
        h0T_host = np.ascontiguousarray(
            (2.0 * h0.T).reshape(4, 128, n).transpose(1, 0, 2)
            .reshape(128, 4 * n)).astype(bft)

        in_maps.append({
            "xT": xT_host,
            "Wxc": Wxc_host,
            "Wc": Wc_host,
            "ATl": AT_host,
            "Af": Af_host,
            "maskT": maskT,
            "id64": id64,
            "injW": injW,
            "onesc": onesc,
            "h0T": h0T_host,
            "s0": (2.0 * h0).astype(np.float32),
            "brow": brow,
        })
    return in_maps


def kernel(x, A, Wx, Wh, Wattn, b):
    from concourse import bass_utils

    nc = _build_kernel()
    in_maps = _prep_host(x, A, Wx, Wh, Wattn, b)
    res = bass_utils.run_bass_kernel_spmd(nc, in_maps, core_ids=list(range(NC)))

    out = np.empty((N, T, H), dtype=np.float32)
    for k in range(NC):
        hs_k = np.asarray(res.results[k]["hso"]).astype(np.float32)  # (T, n, H)
        out[n * k:n * (k + 1)] = 0.5 * hs_k.transpose(1, 0, 2)
    return out
